# revision 8
# baseline (speedup 1.0000x reference)
"""Trainium2 Bass kernel for nn_NodeAttDiff (segment-reduce node attention).

Math (reference):
    e1, e2 = out_gnn[:N], out_gnn[N:]          # N = 200000, D = 256
    diff   = e1 - e2
    h      = relu([e1 e2 diff] @ W1 + b1)      # folded: e1@WA + e2@WB, WA=W1a+W1c, WB=W1b-W1c
    raw    = (h @ W2 + b2)[:, 0]
    att    = segment_softmax(raw, batch)       # 512 contiguous segments (batch sorted)
    out    = segment_sum(att[:,None] * diff)   # [512, 256]

Device strategy (8 cores, graph-partitioned data parallel):
    - 64 graphs / core; each core gets its contiguous node slice (padded to a
      common capacity, pad nodes carry out-of-range segment id -> dropped).
    - Softmax max-subtraction is skipped (raw is O(5); exp is safe in fp32) and
      normalization is algebraic:  out_g = (sum_n w_n diff_n) / (sum_n w_n),
      w_n = exp(raw_n + b2)  -- so no per-node att materialization is needed.
    - Host pre-transposes e1/e2 to feature-major [2,128,cap] and pre-rounds all
      PE operands to tf32 (float32r): f32r matmuls run ~3x faster than fp32 on
      TRN2 and are exact for pre-rounded inputs.
    - Per 512-node tile on device:
        z^T   = WA.T @ e1T + WB.T @ e2T                  (8 f32r matmuls, PSUM)
        h^T   = relu(z^T + b1)                           (ACT, PSUM->SBUF)
        raw   = W2.T @ h^T                               (2 matmuls -> [1,512])
        ew    = exp(raw + b2)                            (ACT -> SBUF row)
        ewT   = I4-trick transpose of ew -> [128,4]      (4 tiny matmuls)
        diffT = e1T - e2T                                (DVE)
        diffN = blockwise PE transpose of diffT          (8 transposes, PSUM)
        w_til = copy diffN -> SBUF                       (DVE + ACT)
        Sw    = (iota == seg_id) * ewT                   (DVE tensor_scalar x4)
        seg  += Sw.T @ [w_til | ones]                    (PSUM accumulate, whole core)
    - Tail: out = seg[:,1:257] * recip(max(seg[:,0:1], eps)), DMA out [64,256].
"""

import os
import numpy as np

NUM_GRAPHS = 512
N_CORES = 8
GPC = NUM_GRAPHS // N_CORES  # graphs per core = 64
D = 256
TILE_N = 512  # nodes per tile
F32R_MASK = np.uint32(0xFFFFE000)


def _tf32_round(x: np.ndarray) -> np.ndarray:
    """Round fp32 to tf32 (10-bit mantissa), round-to-nearest-even."""
    u = np.ascontiguousarray(x, dtype=np.float32).view(np.uint32)
    bias = ((u >> np.uint32(13)) & np.uint32(1)) + np.uint32(0x0FFF)
    return ((u + bias) & F32R_MASK).view(np.float32)


_CACHE = {}


def _build_program(cap: int):
    """Build + compile the SPMD Bass program for per-core node capacity `cap`."""
    if cap in _CACHE:
        return _CACHE[cap]

    from contextlib import ExitStack
    import concourse.bass as bass
    import concourse.tile as tile
    import concourse.bacc as bacc
    import concourse.mybir as mybir

    f32 = mybir.dt.float32
    f32r = mybir.dt.float32r
    AF = mybir.ActivationFunctionType
    ALU = mybir.AluOpType

    n_tiles = cap // TILE_N
    n_cols = cap // 128  # bm columns

    nc = bacc.Bacc("TRN2", target_bir_lowering=False, debug=False,
                   num_devices=N_CORES)

    e1t_d = nc.dram_tensor("e1t", [2, 128, cap], f32r, kind="ExternalInput").ap()
    e2t_d = nc.dram_tensor("e2t", [2, 128, cap], f32r, kind="ExternalInput").ap()
    bm_d = nc.dram_tensor("bm", [128, n_cols], f32, kind="ExternalInput").ap()
    wa_d = nc.dram_tensor("wa", [2, 2, 128, 128], f32r, kind="ExternalInput").ap()
    wb_d = nc.dram_tensor("wb", [2, 2, 128, 128], f32r, kind="ExternalInput").ap()
    w2_d = nc.dram_tensor("w2", [2, 128, 2], f32r, kind="ExternalInput").ap()
    b1_d = nc.dram_tensor("b1", [2, 128, 1], f32, kind="ExternalInput").ap()
    b2_d = nc.dram_tensor("b2", [1, 1], f32, kind="ExternalInput").ap()
    iota_d = nc.dram_tensor("iota", [128, GPC], f32, kind="ExternalInput").ap()
    i128_d = nc.dram_tensor("i128", [128, 128], f32r, kind="ExternalInput").ap()
    i4_d = nc.dram_tensor("i4", [1, 16], f32r, kind="ExternalInput").ap()
    ones_d = nc.dram_tensor("ones", [128, 2], f32r, kind="ExternalInput").ap()
    out_d = nc.dram_tensor("out", [GPC, D], f32, kind="ExternalOutput").ap()

    with tile.TileContext(nc) as tc:
        with ExitStack() as ctx:
            consts = ctx.enter_context(tc.tile_pool(name="consts", bufs=1))
            epool = ctx.enter_context(tc.tile_pool(name="epool", bufs=3))
            dpool = ctx.enter_context(tc.tile_pool(name="dpool", bufs=2))
            hpool = ctx.enter_context(tc.tile_pool(name="hpool", bufs=2))
            spool = ctx.enter_context(tc.tile_pool(name="spool", bufs=2))
            zpool = ctx.enter_context(
                tc.tile_pool(name="zpool", bufs=1, space=bass.MemorySpace.PSUM))
            rpool = ctx.enter_context(
                tc.tile_pool(name="rpool", bufs=2, space=bass.MemorySpace.PSUM))
            etpool = ctx.enter_context(
                tc.tile_pool(name="etpool", bufs=1, space=bass.MemorySpace.PSUM))
            dnpool = ctx.enter_context(
                tc.tile_pool(name="dnpool", bufs=2, space=bass.MemorySpace.PSUM))
            segpool = ctx.enter_context(
                tc.tile_pool(name="segpool", bufs=1, space=bass.MemorySpace.PSUM))

            # ---- constants ----
            wa = consts.tile([128, 2, 2, 128], f32r, tag="wa")
            wb = consts.tile([128, 2, 2, 128], f32r, tag="wb")
            w2 = consts.tile([128, 2, 2], f32r, tag="w2")
            b1 = consts.tile([128, 2, 1], f32, tag="b1")
            b2 = consts.tile([1, 1], f32, tag="b2")
            iota = consts.tile([128, GPC], f32, tag="iota")
            i128 = consts.tile([128, 128], f32r, tag="i128")
            i4 = consts.tile([1, 16], f32r, tag="i4")
            ones = consts.tile([128, 2], f32r, tag="ones")
            bm = consts.tile([128, n_cols], f32, tag="bm")
            # wa/wb dram are [ki, mo, 128, 128]; partition dim must be the
            # 128-row axis -> rearrange to [128, ki, mo, 128]
            nc.sync.dma_start(wa[:], wa_d.rearrange("k m p n -> p k m n"))
            nc.sync.dma_start(wb[:], wb_d.rearrange("k m p n -> p k m n"))
            nc.sync.dma_start(w2[:], w2_d.rearrange("m p n -> p m n"))
            nc.sync.dma_start(b1[:], b1_d.rearrange("m p n -> p m n"))
            nc.sync.dma_start(b2[:], b2_d[:])
            nc.sync.dma_start(iota[:], iota_d[:])
            nc.sync.dma_start(i128[:], i128_d[:])
            nc.sync.dma_start(i4[:], i4_d[:])
            nc.sync.dma_start(ones[:], ones_d[:])
            nc.sync.dma_start(bm[:], bm_d[:])

            seg = segpool.tile([GPC, 2 + D], f32, tag="seg")

            for t in range(n_tiles):
                sl = bass.ts(t, TILE_N)
                e1 = epool.tile([128, 2, TILE_N], f32r, tag="e1")
                e2 = epool.tile([128, 2, TILE_N], f32r, tag="e2")
                nc.sync.dma_start(e1[:], e1t_d[:, :, sl].rearrange("k p n -> p k n"))
                nc.sync.dma_start(e2[:], e2t_d[:, :, sl].rearrange("k p n -> p k n"))

                # z^T [128, mo, 512] accumulated over 4 matmuls per mo-chunk
                z = zpool.tile([128, 2, TILE_N], f32, tag="z")
                for m in range(2):
                    nc.tensor.matmul(z[:, m, :], wa[:, 0, m, :], e1[:, 0, :],
                                     start=True, stop=False)
                    nc.tensor.matmul(z[:, m, :], wa[:, 1, m, :], e1[:, 1, :],
                                     start=False, stop=False)
                    nc.tensor.matmul(z[:, m, :], wb[:, 0, m, :], e2[:, 0, :],
                                     start=False, stop=False)
                    nc.tensor.matmul(z[:, m, :], wb[:, 1, m, :], e2[:, 1, :],
                                     start=False, stop=True)

                # h^T = relu(z + b1)  (ACT, PSUM -> SBUF)
                h = hpool.tile([128, 2, TILE_N], f32r, tag="h")
                for m in range(2):
                    nc.scalar.activation(h[:, m, :], z[:, m, :], AF.Relu,
                                         bias=b1[:, m, :], scale=1.0)

                # raw = W2.T @ h  -> [1, 512] PSUM
                raw = rpool.tile([2, TILE_N], f32, tag="raw")
                nc.tensor.matmul(raw[:], w2[:, 0, :], h[:, 0, :],
                                 start=True, stop=False)
                nc.tensor.matmul(raw[:], w2[:, 1, :], h[:, 1, :],
                                 start=False, stop=True)

                # ew = exp(raw + b2) -> SBUF row [1, 512]
                ew = spool.tile([1, TILE_N], f32r, tag="ew")
                nc.scalar.activation(ew[:], raw[0:1, :], AF.Exp, bias=b2[:], scale=1.0)

                # ewT [128, 4]: 4 outer-product matmuls against I4 rows
                ewt_ps = etpool.tile([128, 4], f32, tag="ewt_ps")
                for b in range(4):
                    nc.tensor.matmul(ewt_ps[:], ew[:, bass.ts(b, 128)],
                                     i4[:, bass.ts(b, 4)],
                                     start=(b == 0), stop=(b == 3))
                ewt = spool.tile([128, 4], f32, tag="ewt")
                nc.vector.tensor_copy(ewt[:], ewt_ps[:])

                # diffT = e1 - e2 (feature-major, f32r)
                dft = dpool.tile([128, 2, TILE_N], f32r, tag="dft")
                nc.vector.tensor_sub(dft[:], e1[:], e2[:])

                # Sw[:, b, :] = (iota == bm_col) * ewt_col
                sw = spool.tile([128, 4, GPC], f32r, tag="sw")
                for b in range(4):
                    nc.vector.tensor_scalar(
                        sw[:, b, :], iota[:], bm[:, 4 * t + b:4 * t + b + 1],
                        ewt[:, b:b + 1], op0=ALU.is_equal, op1=ALU.mult)

                # blockwise transpose diffT -> node-major, 2 blocks per PSUM bank
                wt = hpool.tile([128, 4, D], f32r, tag="wt")
                for half in range(2):
                    dn = dnpool.tile([128, 2, D], f32r, tag="dn")
                    for j in range(2):
                        b = 2 * half + j
                        for k in range(2):
                            nc.tensor.matmul(
                                dn[:, j, bass.ts(k, 128)],
                                dft[:, k, bass.ts(b, 128)], i128[:],
                                is_transpose=True,
                                start=(j == 0 and k == 0),
                                stop=(j == 1 and k == 1),
                            )
                    if half == 0:
                        nc.vector.tensor_copy(wt[:, 0:2, :], dn[:])
                    else:
                        nc.scalar.copy(wt[:, 2:4, :], dn[:])

                # segment accumulate: seg[:, 1:] += Sw_b.T @ diff_b ; seg[:, 0] += Sw_b.T @ 1
                for b in range(4):
                    first = (t == 0 and b == 0)
                    nc.tensor.matmul(seg[:, 2:2 + D], sw[:, b, :], wt[:, b, :],
                                     start=first, stop=False, skip_group_check=True)
                    nc.tensor.matmul(seg[:, 0:2], sw[:, b, :], ones[:],
                                     start=False, stop=(t == n_tiles - 1 and b == 3),
                                     skip_group_check=True)

            # tail: out = seg[:, 1:] / max(seg[:, 0], eps)
            ssum = spool.tile([GPC, 1], f32, tag="ssum")
            nc.vector.tensor_scalar_max(ssum[:], seg[:, 0:1], 1e-30)
            rec = spool.tile([GPC, 1], f32, tag="rec")
            nc.vector.reciprocal(rec[:], ssum[:])
            ot = spool.tile([GPC, D], f32, tag="ot")
            nc.vector.tensor_scalar_mul(ot[:], seg[:, 2:2 + D], rec[:])
            nc.sync.dma_start(out_d[:], ot[:])

    nc.compile()
    _CACHE[cap] = nc
    return nc


def kernel(out_gnn, batch_input, W1, b1, W2, b2):
    import concourse.bass_utils as bass_utils

    out_gnn = np.asarray(out_gnn, dtype=np.float32)
    batch = np.asarray(batch_input, dtype=np.int64)
    W1 = np.asarray(W1, dtype=np.float32)
    b1 = np.asarray(b1, dtype=np.float32)
    W2 = np.asarray(W2, dtype=np.float32)
    b2 = np.asarray(b2, dtype=np.float32)

    half = out_gnn.shape[0] // 2
    batch = batch[:half]
    e1_all, e2_all = out_gnn[:half], out_gnn[half:]

    # per-core contiguous node ranges (graph-partitioned)
    counts = np.bincount(batch, minlength=NUM_GRAPHS)
    bounds = np.zeros(N_CORES + 1, dtype=np.int64)
    bounds[1:] = np.cumsum(counts.reshape(N_CORES, GPC).sum(axis=1))
    max_n = int((bounds[1:] - bounds[:-1]).max())
    cap = max(TILE_N, ((max_n + TILE_N - 1) // TILE_N) * TILE_N)

    nc = _build_program(cap)

    # host-folded MLP weights (fp64 for exactness, then tf32)
    W1a = W1[0:D].astype(np.float64)
    W1b = W1[D:2 * D].astype(np.float64)
    W1c = W1[2 * D:3 * D].astype(np.float64)
    WA = (W1a + W1c).astype(np.float32)
    WB = (W1b - W1c).astype(np.float32)

    def chunk4(w):  # [256,256] -> [ki, mo, 128, 128]
        return np.ascontiguousarray(
            _tf32_round(w).reshape(2, 128, 2, 128).transpose(0, 2, 1, 3))

    common = {
        "wa": chunk4(WA),
        "wb": chunk4(WB),
        "w2": np.ascontiguousarray(np.concatenate([_tf32_round(W2).reshape(2, 128, 1), np.zeros((2, 128, 1), np.float32)], axis=2)),
        "b1": np.ascontiguousarray(b1.reshape(2, 128, 1)),
        "b2": b2.reshape(1, 1).astype(np.float32),
        "iota": np.broadcast_to(np.arange(GPC, dtype=np.float32), (128, GPC)).copy(),
        "i128": np.eye(128, dtype=np.float32),
        "i4": np.eye(4, dtype=np.float32).reshape(1, 16),
        "ones": np.ones((128, 2), dtype=np.float32),
    }

    in_maps = []
    for c in range(N_CORES):
        s, e = int(bounds[c]), int(bounds[c + 1])
        n_c = e - s
        e1t = np.zeros((2, 128, cap), dtype=np.float32)
        e2t = np.zeros((2, 128, cap), dtype=np.float32)
        e1t[:, :, :n_c] = _tf32_round(e1_all[s:e]).T.reshape(2, 128, n_c)
        e2t[:, :, :n_c] = _tf32_round(e2_all[s:e]).T.reshape(2, 128, n_c)
        bmv = np.full(cap, 999.0, dtype=np.float32)
        bmv[:n_c] = (batch[s:e] - GPC * c).astype(np.float32)
        in_maps.append({
            "e1t": e1t, "e2t": e2t,
            "bm": np.ascontiguousarray(bmv.reshape(cap // 128, 128).T),
            **common,
        })

    trace_dir = os.environ.get("NODEATT_TRACE_DIR")
    kw = {}
    if trace_dir:
        kw = {"trace": True, "tmpdir": trace_dir}
    res = bass_utils.run_bass_kernel_spmd(
        nc, in_maps, core_ids=list(range(N_CORES)), **kw)
    if trace_dir:
        kernel.last_exec_time_ns = res.exec_time_ns
        kernel.last_results = res

    return np.concatenate([res.results[c]["out"] for c in range(N_CORES)], axis=0)


# revision 18
# speedup vs baseline: 1.8864x; 1.8864x over previous
"""Trainium2 Bass kernel for nn_NodeAttDiff (segment-reduce node attention).

Math (reference):
    e1, e2 = out_gnn[:N], out_gnn[N:]          # N = 200000, D = 256
    diff   = e1 - e2
    h      = relu([e1 e2 diff] @ W1 + b1)      # folded: e1@WA + e2@WB, WA=W1a+W1c, WB=W1b-W1c
    raw    = (h @ W2 + b2)[:, 0]
    att    = segment_softmax(raw, batch)       # 512 contiguous segments (batch sorted)
    out    = segment_sum(att[:,None] * diff)   # [512, 256]

Device strategy (8 cores, graph-partitioned data parallel):
    - 64 graphs / core; each core gets its contiguous node slice (padded to a
      common capacity, pad nodes carry out-of-range segment id -> dropped).
    - Softmax max-subtraction is skipped (raw is O(5); exp is safe in fp32) and
      normalization is algebraic:  out_g = (sum_n w_n diff_n) / (sum_n w_n),
      w_n = exp(raw_n + b2)  -- so no per-node att materialization is needed.
    - Host pre-transposes e1/e2 to feature-major [2,128,cap] and pre-rounds all
      PE operands to tf32 (float32r): f32r matmuls run ~3x faster than fp32 on
      TRN2 and are exact for pre-rounded inputs.
    - Per 512-node tile on device:
        z^T   = WA.T @ e1T + WB.T @ e2T                  (8 f32r matmuls, PSUM)
        h^T   = relu(z^T + b1)                           (ACT, PSUM->SBUF)
        raw   = W2.T @ h^T                               (2 matmuls -> [1,512])
        ew    = exp(raw + b2)                            (ACT -> SBUF row)
        ewT   = I4-trick transpose of ew -> [128,4]      (4 tiny matmuls)
        diffT = e1T - e2T                                (DVE)
        diffN = blockwise PE transpose of diffT          (8 transposes, PSUM)
        w_til = copy diffN -> SBUF                       (DVE + ACT)
        Sw    = (iota == seg_id) * ewT                   (DVE tensor_scalar x4)
        seg  += Sw.T @ [w_til | ones]                    (PSUM accumulate, whole core)
    - Tail: out = seg[:,1:257] * recip(max(seg[:,0:1], eps)), DMA out [64,256].
"""

import os
import numpy as np

NUM_GRAPHS = 512
N_CORES = 8
GPC = 64  # graph-window width per core (one-hot columns); set per-input at build
D = 256
TILE_N = 512  # nodes per tile
F32R_MASK = np.uint32(0xFFFFE000)


def _tf32_round(x: np.ndarray) -> np.ndarray:
    """Round fp32 to tf32 (10-bit mantissa), round-to-nearest-even."""
    u = np.ascontiguousarray(x, dtype=np.float32).view(np.uint32)
    bias = ((u >> np.uint32(13)) & np.uint32(1)) + np.uint32(0x0FFF)
    return ((u + bias) & F32R_MASK).view(np.float32)


_CACHE = {}


def _build_program(cap: int, gw: int):
    """Build + compile the SPMD Bass program; `cap` nodes and a `gw`-graph
    window per core."""
    if (cap, gw) in _CACHE:
        return _CACHE[(cap, gw)]

    from contextlib import ExitStack
    import concourse.bass as bass
    import concourse.tile as tile
    import concourse.bacc as bacc
    import concourse.mybir as mybir

    f32 = mybir.dt.float32
    f32r = mybir.dt.float32r
    AF = mybir.ActivationFunctionType
    ALU = mybir.AluOpType

    n_tiles = cap // TILE_N
    n_cols = cap // 128  # bm columns

    nc = bacc.Bacc("TRN2", target_bir_lowering=False, debug=False,
                   num_devices=N_CORES)

    e1t_d = nc.dram_tensor("e1t", [2, 128, cap], f32r, kind="ExternalInput").ap()
    e2t_d = nc.dram_tensor("e2t", [2, 128, cap], f32r, kind="ExternalInput").ap()
    bm_d = nc.dram_tensor("bm", [128, n_cols], f32, kind="ExternalInput").ap()
    wa_d = nc.dram_tensor("wa", [2, 2, 128, 128], f32r, kind="ExternalInput").ap()
    wb_d = nc.dram_tensor("wb", [2, 2, 128, 128], f32r, kind="ExternalInput").ap()
    w2_d = nc.dram_tensor("w2", [2, 128, 2], f32r, kind="ExternalInput").ap()
    b1_d = nc.dram_tensor("b1", [2, 128, 1], f32, kind="ExternalInput").ap()
    b2_d = nc.dram_tensor("b2", [1, 1], f32, kind="ExternalInput").ap()
    iota_d = nc.dram_tensor("iota", [128, gw], f32, kind="ExternalInput").ap()
    i128_d = nc.dram_tensor("i128", [128, 128], f32r, kind="ExternalInput").ap()
    i4_d = nc.dram_tensor("i4", [1, 16], f32r, kind="ExternalInput").ap()
    ones_d = nc.dram_tensor("ones", [128, 2], f32r, kind="ExternalInput").ap()
    out_d = nc.dram_tensor("out", [gw, D], f32, kind="ExternalOutput").ap()

    with tile.TileContext(nc) as tc:
        with ExitStack() as ctx:
            consts = ctx.enter_context(tc.tile_pool(name="consts", bufs=1))
            epool = ctx.enter_context(tc.tile_pool(name="epool", bufs=3))
            dpool = ctx.enter_context(tc.tile_pool(name="dpool", bufs=2))
            hpool = ctx.enter_context(tc.tile_pool(name="hpool", bufs=2))
            spool = ctx.enter_context(tc.tile_pool(name="spool", bufs=2))
            zpool = ctx.enter_context(
                tc.tile_pool(name="zpool", bufs=1, space=bass.MemorySpace.PSUM))
            rpool = ctx.enter_context(
                tc.tile_pool(name="rpool", bufs=2, space=bass.MemorySpace.PSUM))
            etpool = ctx.enter_context(
                tc.tile_pool(name="etpool", bufs=1, space=bass.MemorySpace.PSUM))
            dnpool = ctx.enter_context(
                tc.tile_pool(name="dnpool", bufs=2, space=bass.MemorySpace.PSUM))
            segpool = ctx.enter_context(
                tc.tile_pool(name="segpool", bufs=1, space=bass.MemorySpace.PSUM))

            # ---- constants ----
            wa = consts.tile([128, 2, 2, 128], f32r, tag="wa")
            wb = consts.tile([128, 2, 2, 128], f32r, tag="wb")
            w2 = consts.tile([128, 2, 2], f32r, tag="w2")
            b1 = consts.tile([128, 2, 1], f32, tag="b1")
            b2 = consts.tile([1, 1], f32, tag="b2")
            iota = consts.tile([128, gw], f32, tag="iota")
            i128 = consts.tile([128, 128], f32r, tag="i128")
            i4 = consts.tile([1, 16], f32r, tag="i4")
            ones = consts.tile([128, 2], f32r, tag="ones")
            bm = consts.tile([128, n_cols], f32, tag="bm")
            # wa/wb dram are [ki, mo, 128, 128]; partition dim must be the
            # 128-row axis -> rearrange to [128, ki, mo, 128]
            nc.sync.dma_start(wa[:], wa_d.rearrange("k m p n -> p k m n"))
            nc.sync.dma_start(wb[:], wb_d.rearrange("k m p n -> p k m n"))
            nc.sync.dma_start(w2[:], w2_d.rearrange("m p n -> p m n"))
            nc.sync.dma_start(b1[:], b1_d.rearrange("m p n -> p m n"))
            nc.sync.dma_start(b2[:], b2_d[:])
            nc.sync.dma_start(iota[:], iota_d[:])
            nc.sync.dma_start(i128[:], i128_d[:])
            nc.sync.dma_start(i4[:], i4_d[:])
            nc.sync.dma_start(ones[:], ones_d[:])
            nc.sync.dma_start(bm[:], bm_d[:])

            seg = segpool.tile([gw, 2 + D], f32, tag="seg")

            for t in range(n_tiles):
                sl = bass.ts(t, TILE_N)
                e1 = epool.tile([128, 2, TILE_N], f32r, tag="e1")
                e2 = epool.tile([128, 2, TILE_N], f32r, tag="e2")
                nc.sync.dma_start(e1[:], e1t_d[:, :, sl].rearrange("k p n -> p k n"))
                nc.sync.dma_start(e2[:], e2t_d[:, :, sl].rearrange("k p n -> p k n"))

                # z^T [128, mo, 512] accumulated over 4 matmuls per mo-chunk
                z = zpool.tile([128, 2, TILE_N], f32, tag="z")
                for m in range(2):
                    nc.tensor.matmul(z[:, m, :], wa[:, 0, m, :], e1[:, 0, :],
                                     start=True, stop=False)
                    nc.tensor.matmul(z[:, m, :], wa[:, 1, m, :], e1[:, 1, :],
                                     start=False, stop=False)
                    nc.tensor.matmul(z[:, m, :], wb[:, 0, m, :], e2[:, 0, :],
                                     start=False, stop=False)
                    nc.tensor.matmul(z[:, m, :], wb[:, 1, m, :], e2[:, 1, :],
                                     start=False, stop=True)

                # h^T = relu(z + b1)  (ACT, PSUM -> SBUF)
                h = hpool.tile([128, 2, TILE_N], f32r, tag="h")
                for m in range(2):
                    nc.scalar.activation(h[:, m, :], z[:, m, :], AF.Relu,
                                         bias=b1[:, m, :], scale=1.0)

                # raw = W2.T @ h  -> [1, 512] PSUM
                raw = rpool.tile([2, TILE_N], f32, tag="raw")
                nc.tensor.matmul(raw[:], w2[:, 0, :], h[:, 0, :],
                                 start=True, stop=False)
                nc.tensor.matmul(raw[:], w2[:, 1, :], h[:, 1, :],
                                 start=False, stop=True)

                # ew = exp(raw + b2) -> SBUF row [1, 512]
                ew = spool.tile([1, TILE_N], f32r, tag="ew")
                nc.scalar.activation(ew[:], raw[0:1, :], AF.Exp, bias=b2[:], scale=1.0)

                # ewT [128, 4]: 4 outer-product matmuls against I4 rows
                ewt_ps = etpool.tile([128, 4], f32, tag="ewt_ps")
                for b in range(4):
                    nc.tensor.matmul(ewt_ps[:], ew[:, bass.ts(b, 128)],
                                     i4[:, bass.ts(b, 4)],
                                     start=(b == 0), stop=(b == 3))
                ewt = spool.tile([128, 4], f32, tag="ewt")
                nc.vector.tensor_copy(ewt[:], ewt_ps[:])

                # diffT = e1 - e2 (feature-major, f32r)
                dft = dpool.tile([128, 2, TILE_N], f32r, tag="dft")
                nc.vector.tensor_sub(dft[:], e1[:], e2[:])

                # Sw[:, b, :] = (iota == bm_col) * ewt_col
                sw = spool.tile([128, 4, gw], f32r, tag="sw")
                for b in range(4):
                    nc.vector.tensor_scalar(
                        sw[:, b, :], iota[:], bm[:, 4 * t + b:4 * t + b + 1],
                        ewt[:, b:b + 1], op0=ALU.is_equal, op1=ALU.mult)

                # blockwise transpose diffT -> node-major, 2 blocks per PSUM bank
                wt = hpool.tile([128, 4, D], f32r, tag="wt")
                for half in range(2):
                    dn = dnpool.tile([128, 2, D], f32r, tag="dn")
                    for j in range(2):
                        b = 2 * half + j
                        for k in range(2):
                            nc.tensor.matmul(
                                dn[:, j, bass.ts(k, 128)],
                                dft[:, k, bass.ts(b, 128)], i128[:],
                                is_transpose=True,
                                start=(j == 0 and k == 0),
                                stop=(j == 1 and k == 1),
                            )
                    if half == 0:
                        nc.vector.tensor_copy(wt[:, 0:2, :], dn[:])
                    else:
                        nc.scalar.copy(wt[:, 2:4, :], dn[:])

                # segment accumulate: seg[:, 1:] += Sw_b.T @ diff_b ; seg[:, 0] += Sw_b.T @ 1
                for b in range(4):
                    first = (t == 0 and b == 0)
                    nc.tensor.matmul(seg[:, 2:2 + D], sw[:, b, :], wt[:, b, :],
                                     start=first, stop=False, skip_group_check=True)
                    nc.tensor.matmul(seg[:, 0:2], sw[:, b, :], ones[:],
                                     start=False, stop=(t == n_tiles - 1 and b == 3),
                                     skip_group_check=True)

            # tail: out = seg[:, 1:] / max(seg[:, 0], eps)
            ssum = spool.tile([gw, 1], f32, tag="ssum")
            nc.vector.tensor_scalar_max(ssum[:], seg[:, 0:1], 1e-30)
            rec = spool.tile([gw, 1], f32, tag="rec")
            nc.vector.reciprocal(rec[:], ssum[:])
            ot = spool.tile([gw, D], f32, tag="ot")
            nc.vector.tensor_scalar_mul(ot[:], seg[:, 2:2 + D], rec[:])
            nc.sync.dma_start(out_d[:], ot[:])

    nc.compile()
    _CACHE[(cap, gw)] = nc
    return nc


def _prepare(out_gnn, batch_input, W1, b1, W2, b2):
    out_gnn = np.asarray(out_gnn, dtype=np.float32)
    batch = np.asarray(batch_input, dtype=np.int64)
    W1 = np.asarray(W1, dtype=np.float32)
    b1 = np.asarray(b1, dtype=np.float32)
    W2 = np.asarray(W2, dtype=np.float32)
    b2 = np.asarray(b2, dtype=np.float32)

    half = out_gnn.shape[0] // 2
    batch = batch[:half]
    e1_all, e2_all = out_gnn[:half], out_gnn[half:]

    # Node-balanced, graph-aligned contiguous cuts. Core c handles graphs
    # [gcut[c], gcut[c+1]) and the matching contiguous node range. The
    # sorted batch may populate only a prefix of the 512 graphs, so cuts
    # are chosen by node mass, not by fixed graph ranges.
    counts = np.bincount(batch, minlength=NUM_GRAPHS)
    ccum = np.concatenate([[0], np.cumsum(counts)])  # node offset per graph
    # only graphs up to the last populated one get device windows; trailing
    # empty graphs stay host-side zeros
    g_used = int(np.max(np.nonzero(counts)[0])) + 1 if counts.any() else 1
    gcut = np.zeros(N_CORES + 1, dtype=np.int64)
    gcut[N_CORES] = g_used
    for c in range(1, N_CORES):
        g = int(np.searchsorted(ccum, ccum[g_used] * c / N_CORES, side="left"))
        gcut[c] = min(max(g, gcut[c - 1]), g_used)
    spans = gcut[1:] - gcut[:-1]
    if spans.max() > 128:
        # node-balanced cuts gave an oversized graph window (pathological
        # distribution) -- fall back to an even graph split of [0, g_used)
        gcut = np.round(np.linspace(0, g_used, N_CORES + 1)).astype(np.int64)
        spans = gcut[1:] - gcut[:-1]
        if spans.max() > 128:
            raise ValueError(f"graph window {spans.max()} > 128 unsupported")

    nbounds = ccum[gcut]  # node boundaries per core
    gw = int(max(2, ((spans.max() + 1) // 2) * 2))
    max_n = int((nbounds[1:] - nbounds[:-1]).max())
    cap = max(TILE_N, ((max_n + TILE_N - 1) // TILE_N) * TILE_N)

    nc = _build_program(cap, gw)

    # host-folded MLP weights (fp64 for exactness, then tf32)
    W1a = W1[0:D].astype(np.float64)
    W1b = W1[D:2 * D].astype(np.float64)
    W1c = W1[2 * D:3 * D].astype(np.float64)
    WA = (W1a + W1c).astype(np.float32)
    WB = (W1b - W1c).astype(np.float32)

    def chunk4(w):  # [256,256] -> [ki, mo, 128, 128]
        return np.ascontiguousarray(
            _tf32_round(w).reshape(2, 128, 2, 128).transpose(0, 2, 1, 3))

    common = {
        "wa": chunk4(WA),
        "wb": chunk4(WB),
        "w2": np.ascontiguousarray(np.concatenate([_tf32_round(W2).reshape(2, 128, 1), np.zeros((2, 128, 1), np.float32)], axis=2)),
        "b1": np.ascontiguousarray(b1.reshape(2, 128, 1)),
        "b2": b2.reshape(1, 1).astype(np.float32),
        "iota": np.broadcast_to(np.arange(gw, dtype=np.float32), (128, gw)).copy(),
        "i128": np.eye(128, dtype=np.float32),
        "i4": np.eye(4, dtype=np.float32).reshape(1, 16),
        "ones": np.ones((128, 2), dtype=np.float32),
    }

    in_maps = []
    for c in range(N_CORES):
        s, e = int(nbounds[c]), int(nbounds[c + 1])
        n_c = e - s
        e1t = np.zeros((2, 128, cap), dtype=np.float32)
        e2t = np.zeros((2, 128, cap), dtype=np.float32)
        e1t[:, :, :n_c] = _tf32_round(e1_all[s:e]).T.reshape(2, 128, n_c)
        e2t[:, :, :n_c] = _tf32_round(e2_all[s:e]).T.reshape(2, 128, n_c)
        bmv = np.full(cap, 999.0, dtype=np.float32)
        bmv[:n_c] = (batch[s:e] - gcut[c]).astype(np.float32)
        in_maps.append({
            "e1t": e1t, "e2t": e2t,
            "bm": np.ascontiguousarray(bmv.reshape(cap // 128, 128).T),
            **common,
        })
    return nc, in_maps, gcut


def kernel(out_gnn, batch_input, W1, b1, W2, b2):
    import concourse.bass_utils as bass_utils

    nc, in_maps, gcut = _prepare(out_gnn, batch_input, W1, b1, W2, b2)

    trace_dir = os.environ.get("NODEATT_TRACE_DIR")
    kw = {}
    if trace_dir:
        kw = {"trace": True, "tmpdir": trace_dir}
    res = bass_utils.run_bass_kernel_spmd(
        nc, in_maps, core_ids=list(range(N_CORES)), **kw)
    if trace_dir:
        kernel.last_exec_time_ns = res.exec_time_ns
        kernel.last_results = res

    out = np.zeros((NUM_GRAPHS, D), dtype=np.float32)
    for c in range(N_CORES):
        span = int(gcut[c + 1] - gcut[c])
        if span > 0:
            out[gcut[c]:gcut[c + 1]] = res.results[c]["out"][:span]
    return out
